# revision 21
# baseline (speedup 1.0000x reference)
"""Bass/Tile kernel for BSplineField3d (tricubic B-spline interpolation).

v2 design (cost-model-driven):
  Table: Cy8[xq=32, yc=125, z=128, xs=8, c=3, ky=4] in bf16 (98 MB DRAM).
    Cy8[xq,yc,z,xs,c,ky] = sum_m A[ky,m] * phi[min(4*xq+xs,127), yc+m, z, c]
    The y-dimension is pre-contracted into a degree-3 polynomial in v
    (coefficient index ky); the 8-wide x-slot span (xs) makes each point's
    full 64-tap data ONE contiguous 768-byte record:
        rec(xq, yc, z0) = Cy8[xq, yc, z0:z0+4, :, :, :]   (384 bf16)
    built with PE matmuls (bf16) against a banded B-spline matrix.
  Phase 2: per chunk of 128xP points:
    - cell indices + fractional coords (DVE/Act)
    - ONE indirect-DMA record per point (1984 gather instructions total,
      each 128 records of 768B, issued on gpsimd)
    - combine on DVE in bf16 (2x packed mode):
        mult by W16[z,ky] = ww[z]*v^ky, tree-reduce z, tree-reduce ky,
        mult by W8c[xs,c] = w8[xs] (masked wu), tree-reduce xs.
"""

from contextlib import ExitStack

import sys as _sys
for _p in ("/opt/trn_rl_repo",):
    if _p not in _sys.path:
        _sys.path.append(_p)

import numpy as np

import concourse.bass as bass
import concourse.tile as tile
from concourse import mybir
from concourse._compat import with_exitstack

F32 = mybir.dt.float32
BF16 = mybir.dt.bfloat16
I32 = mybir.dt.int32

NX = 128          # grid points per dim
NCELL = 125       # valid cells per dim (ix in [0,124])
NC_ = 3           # components
ZC = NX * NC_     # 384 floats per (y,x) z-row in transposed phi
UNIT = 96         # bf16 elems per (xq,yc,z): [xs8, c3, ky4]
RECE = 4 * UNIT   # 384 elems per record (z-window of 4 units)
NXQ = 32
TAB_ELEMS = NXQ * NCELL * NX * UNIT  # 49,152,000

COLS = 1984       # points per partition (128*1984 = 253952 >= 250000)
P = 64            # points per partition per chunk
NCHUNK = COLS // P  # 31

INV_D = 62.5      # 1/dx, dx = 2/125


def bspline_poly_A():
    """A[k][m]: coefficient of v^k in the cubic B-spline weight of tap m."""
    return np.array(
        [
            [1 / 6, 4 / 6, 1 / 6, 0.0],
            [-3 / 6, 0.0, 3 / 6, 0.0],
            [3 / 6, -6 / 6, 3 / 6, 0.0],
            [-1 / 6, 3 / 6, -3 / 6, 1 / 6],
        ],
        dtype=np.float64,
    )


def build_W_const():
    """W[y, ky*125+yc] = A[ky, y-yc] for 0 <= y-yc <= 3 else 0; bf16 [128, 500]."""
    import ml_dtypes
    A = bspline_poly_A()
    W = np.zeros((128, 4, 125), np.float32)
    for yc in range(NCELL):
        for m in range(4):
            for k in range(4):
                W[yc + m, k, yc] = A[k, m]
    return W.reshape(128, 500).astype(ml_dtypes.bfloat16)


def _ap(t, offset, dims):
    """Raw AP on the same tensor as AP `t` with explicit [step, num] dims."""
    return bass.AP(tensor=t.tensor, offset=t.offset + offset, ap=[list(d) for d in dims])


@with_exitstack
def bspline_kernel(ctx: ExitStack, tc: tile.TileContext, outs, ins):
    """outs = [T_out [128, COLS, 3] f32]
    ins  = [xs, ys, zs [128, COLS] f32, phi_t [128, 49152] bf16 (y-major)]"""
    nc = tc.nc
    xs, ys, zs, phi = ins
    t_out = outs[0]

    w_dram = nc.inline_tensor(build_W_const(), name="w_const")

    dram = ctx.enter_context(tc.tile_pool(name="cydram", bufs=1, space="DRAM"))
    cy = dram.tile([NXQ, NCELL, NX * UNIT], BF16)

    # ---------------- Phase 1: build Cy8 ----------------
    with ExitStack() as p1:
        singles = p1.enter_context(tc.tile_pool(name="p1_singles", bufs=1))
        stages = p1.enter_context(tc.tile_pool(name="p1_stage", bufs=1))
        psums = p1.enter_context(tc.psum_pool(name="p1_psum", bufs=2))

        w_sb = singles.tile([128, 500], BF16)
        nc.sync.dma_start(out=w_sb[:], in_=w_dram.ap())
        phi_sb = singles.tile([128, 128 * ZC], BF16)
        # 4 loads so they pipeline with the first matmuls
        for q in range(4):
            nc.sync.dma_start(
                out=phi_sb[:, q * 32 * ZC:(q + 1) * 32 * ZC],
                in_=_ap(phi, q * 32 * ZC, [[128 * ZC, 128], [1, 32 * ZC]]),
            )

        # per x: one psum [125, 4*512] (4 banks, one per ky); then one strided
        # copy per destination stage slot covering all (z, c, ky) at once.
        cp_engines = [nc.scalar, nc.vector]
        wr_engines = [nc.sync, nc.sync, nc.scalar, nc.gpsimd]
        stage_tiles = {}
        cp_i = 0

        def emit_copy(ps, stage, slot):
            nonlocal cp_i
            # psum [yc, ky*512 + z*3 + c] f32 -> stage[yc, z*96 + slot*12 + c*4 + ky]
            src = _ap(ps[:], 0, [[4 * 512, NCELL], [3, NX], [1, NC_], [512, 4]])
            dst = _ap(stage[:], slot * 12,
                      [[NX * UNIT, NCELL], [UNIT, NX], [4, NC_], [1, 4]])
            eng = cp_engines[cp_i % len(cp_engines)]
            cp_i += 1
            if eng is nc.scalar:
                eng.copy(out=dst, in_=src)
            else:
                eng.tensor_copy(out=dst, in_=src)

        for x in range(NX):
            ps = psums.tile([NCELL, 4 * 512], F32)
            for ky in range(4):
                nc.tensor.matmul(
                    ps[:, ky * 512:ky * 512 + ZC],
                    w_sb[:, ky * NCELL:(ky + 1) * NCELL],
                    phi_sb[:, x * ZC:(x + 1) * ZC],
                    start=True,
                    stop=True,
                )
            xq_a, s_a = x // 4, x % 4
            if xq_a not in stage_tiles:
                stage_tiles[xq_a] = stages.tile([128, NX * UNIT], BF16,
                                                name=f"stage{xq_a % 3}")
            emit_copy(ps, stage_tiles[xq_a], s_a)
            if xq_a >= 1:
                emit_copy(ps, stage_tiles[xq_a - 1], s_a + 4)
            if x == NX - 1:
                # records of xq=31 cover x=124..131; fill slots 4..7 with x=127
                for s in range(4, 8):
                    emit_copy(ps, stage_tiles[xq_a], s)
            # stage xq complete once x = 4*xq+7 processed (or final x)
            done = []
            for xq, st in stage_tiles.items():
                if x == NX - 1 or x == 4 * xq + 7:
                    wr_engines[xq % len(wr_engines)].dma_start(
                        out=cy[xq, :, :], in_=st[:NCELL, :])
                    done.append(xq)
            for xq in done:
                del stage_tiles[xq]

    # ---------------- Phase 2: points ----------------
    cy_flat = _ap(cy[:], 0, [[TAB_ELEMS, 1], [1, TAB_ELEMS]])

    with ExitStack() as p2:
        sing = p2.enter_context(tc.tile_pool(name="p2_singles", bufs=1))
        coords = p2.enter_context(tc.tile_pool(name="p2_coords", bufs=3))
        small = p2.enter_context(tc.tile_pool(name="p2_small", bufs=2))
        idxp = p2.enter_context(tc.tile_pool(name="p2_idx", bufs=3))
        recs = p2.enter_context(tc.tile_pool(name="p2_rec", bufs=2))
        prods = p2.enter_context(tc.tile_pool(name="p2_prod", bufs=1))
        touts = p2.enter_context(tc.tile_pool(name="p2_tout", bufs=2))

        # j-ramp constant: [128, 8] = 0..7 (x-slot index within record)
        jr8f = sing.tile([128, 8], F32)
        nc.gpsimd.iota(jr8f[:], [[1, 8]], channel_multiplier=0,
                       allow_small_or_imprecise_dtypes=True)
        jr8 = sing.tile([128, 8], BF16)
        nc.scalar.copy(out=jr8[:], in_=jr8f[:])

        for ch in range(NCHUNK):
            x_t = coords.tile([128, P], F32)
            y_t = coords.tile([128, P], F32)
            z_t = coords.tile([128, P], F32)
            nc.sync.dma_start(out=x_t[:], in_=xs[:, ch * P:(ch + 1) * P])
            nc.sync.dma_start(out=y_t[:], in_=ys[:, ch * P:(ch + 1) * P])
            nc.sync.dma_start(out=z_t[:], in_=zs[:, ch * P:(ch + 1) * P])

            # --- cell indices + fractions ---
            def exact_floor(src, out, sfx):
                # out = floor(src) for src >= 0; f32<->i32 converts round to
                # nearest, so correct with an is_gt mask. Converts on Act.
                ci = small.tile([128, P], I32, name=f"ci_{sfx}")
                cf = small.tile([128, P], F32, name=f"cf_{sfx}")
                nc.scalar.copy(out=ci[:], in_=src[:])
                nc.scalar.copy(out=cf[:], in_=ci[:])
                nc.vector.tensor_tensor(out[:], cf[:], src[:], mybir.AluOpType.is_gt)
                nc.vector.tensor_sub(out[:], cf[:], out[:])

            def split_coord(src, sfx):
                u = small.tile([128, P], F32, name=f"u_{sfx}")
                fr = small.tile([128, P], F32, name=f"fr_{sfx}")
                ii = small.tile([128, P], F32, name=f"ii_{sfx}")
                # u = (x+1)*62.5 on Act
                nc.scalar.activation(u[:], src[:],
                                     mybir.ActivationFunctionType.Copy,
                                     bias=INV_D, scale=INV_D)
                exact_floor(u, ii, sfx)
                nc.vector.tensor_scalar(ii[:], ii[:], float(NCELL - 1), 0.0,
                                        mybir.AluOpType.min, mybir.AluOpType.max)
                nc.vector.tensor_sub(fr[:], u[:], ii[:])
                return ii, fr

            ix_t, fu = split_coord(x_t, "x")
            iy_t, fv = split_coord(y_t, "y")
            iz_t, fw = split_coord(z_t, "z")

            # --- x-quad decomposition: xq = ix//4, s0 = ix%4 ---
            tq = small.tile([128, P], F32)
            xq_t = small.tile([128, P], F32)
            s0_t = small.tile([128, P], F32)
            nc.vector.tensor_scalar(tq[:], ix_t[:], 0.25, None, mybir.AluOpType.mult)
            exact_floor(tq, xq_t, "q")
            nc.vector.scalar_tensor_tensor(
                s0_t[:], xq_t[:], -4.0, ix_t[:],
                mybir.AluOpType.mult, mybir.AluOpType.add)

            # --- record index (elem units): ((xq*125+yc)*128+z0)*96
            #     = 32 * (3 * (xq*16000 + yc*128 + z0))
            byz = small.tile([128, P], F32)
            nc.vector.scalar_tensor_tensor(
                byz[:], iy_t[:], float(NX), iz_t[:],
                mybir.AluOpType.mult, mybir.AluOpType.add)
            idx_f = small.tile([128, P], F32)
            nc.vector.scalar_tensor_tensor(
                idx_f[:], xq_t[:], 16000.0, byz[:],
                mybir.AluOpType.mult, mybir.AluOpType.add)
            nc.vector.tensor_scalar(idx_f[:], idx_f[:], 3.0, None,
                                    mybir.AluOpType.mult)
            idx_i = idxp.tile([128, P], I32)
            nc.vector.tensor_copy(out=idx_i[:], in_=idx_f[:])
            nc.vector.tensor_scalar(idx_i[:], idx_i[:], 32, None,
                                    mybir.AluOpType.mult)

            # --- tap weights ---
            def tap_weights(fr, sfx):
                wt = small.tile([128, P, 4], F32, name=f"wt_{sfx}")
                t = small.tile([128, P], F32, name=f"t_{sfx}")
                t2 = small.tile([128, P], F32, name=f"t2_{sfx}")
                r2 = small.tile([128, P], F32, name=f"r2_{sfx}")
                r3 = small.tile([128, P], F32, name=f"r3_{sfx}")
                w0 = wt[:, :, 0]
                w1 = wt[:, :, 1]
                w2 = wt[:, :, 2]
                w3 = wt[:, :, 3]
                nc.vector.tensor_scalar(t[:], fr[:], -1.0, 1.0,
                                        mybir.AluOpType.mult, mybir.AluOpType.add)
                nc.scalar.square(t2[:], t[:])
                nc.vector.scalar_tensor_tensor(w0, t2[:], 1 / 6, t[:],
                                               mybir.AluOpType.mult, mybir.AluOpType.mult)
                nc.scalar.square(r2[:], fr[:])
                nc.vector.tensor_mul(r3[:], r2[:], fr[:])
                nc.vector.tensor_scalar(w3, r3[:], 1 / 6, None, mybir.AluOpType.mult)
                nc.vector.scalar_tensor_tensor(w1, r3[:], 0.5, r2[:],
                                               mybir.AluOpType.mult, mybir.AluOpType.subtract)
                nc.vector.tensor_scalar(w1, w1, 2 / 3, None, mybir.AluOpType.add)
                nc.vector.tensor_add(w2, w0, w1)
                nc.vector.tensor_add(w2, w2, w3)
                nc.vector.tensor_scalar(w2, w2, -1.0, 1.0,
                                        mybir.AluOpType.mult, mybir.AluOpType.add)
                return wt

            wu = tap_weights(fu, "u")
            ww = tap_weights(fw, "w")

            vp = small.tile([128, P, 4], F32)
            nc.vector.memset(vp[:, :, 0], 1.0)
            nc.vector.tensor_copy(out=vp[:, :, 1], in_=fv[:])
            nc.scalar.square(vp[:, :, 2], fv[:])
            nc.vector.tensor_mul(vp[:, :, 3], vp[:, :, 2], fv[:])

            # --- W16[pt, z4, ky4] = ww[z] * v^ky  (bf16) ---
            w16 = small.tile([128, P, 16], BF16)
            nc.vector.tensor_tensor(
                _ap(w16[:], 0, [[P * 16, 128], [16, P], [4, 4], [1, 4]]),
                _ap(ww[:], 0, [[P * 4, 128], [4, P], [1, 4], [0, 4]]),
                _ap(vp[:], 0, [[P * 4, 128], [4, P], [0, 4], [1, 4]]),
                mybir.AluOpType.mult)

            # --- w8[pt, j] = wu[j - s0] for j-s0 in [0,4) else 0 ---
            # whole chain in bf16 (d8/e8 are small ints / 0-1 masks, exact in
            # bf16; wu is rounded to bf16 exactly once, same as v2's W8c)
            s0b = small.tile([128, P], BF16)
            wub = small.tile([128, P, 4], BF16)
            nc.scalar.copy(out=s0b[:], in_=s0_t[:])
            nc.scalar.copy(out=wub[:], in_=wu[:])
            d8 = small.tile([128, P, 8], BF16)
            e8 = small.tile([128, P, 8], BF16)
            w8 = small.tile([128, P, 8], BF16)
            nc.vector.tensor_tensor(
                _ap(d8[:], 0, [[P * 8, 128], [8, P], [1, 8]]),
                _ap(jr8[:], 0, [[8, 128], [0, P], [1, 8]]),
                _ap(s0b[:], 0, [[P, 128], [1, P], [0, 8]]),
                mybir.AluOpType.subtract)
            for l in range(4):
                tgt = w8 if l == 0 else e8
                nc.vector.tensor_scalar(e8[:], d8[:], float(l), None,
                                        mybir.AluOpType.is_equal)
                nc.vector.tensor_tensor(
                    _ap(tgt[:], 0, [[P * 8, 128], [8, P], [1, 8]]),
                    _ap(e8[:], 0, [[P * 8, 128], [8, P], [1, 8]]),
                    _ap(wub[:], l, [[P * 4, 128], [4, P], [0, 8]]),
                    mybir.AluOpType.mult)
                if l > 0:
                    nc.vector.tensor_add(w8[:], w8[:], e8[:])

            # --- W8c[pt, xs8, c3] = w8[xs] replicated over c (bf16, Act) ---
            w8c = small.tile([128, P, 24], BF16)
            nc.scalar.copy(
                out=_ap(w8c[:], 0, [[P * 24, 128], [24, P], [3, 8], [1, 3]]),
                in_=_ap(w8[:], 0, [[P * 8, 128], [8, P], [1, 8], [0, 3]]))

            # --- gather: one 768B record per point ---
            rec = recs.tile([128, P * RECE], BF16)
            for t in range(P):
                nc.gpsimd.indirect_dma_start(
                    out=_ap(rec[:], t * RECE, [[P * RECE, 128], [1, RECE]]),
                    out_offset=None,
                    in_=cy_flat,
                    in_offset=bass.IndirectOffsetOnAxis(
                        ap=_ap(idx_i[:], t, [[P, 128], [1, 1]]), axis=1),
                )

            # --- combine ---
            # rec[pt, z4, xs8, c3, ky4]; iteration ((pt,z) merged, xsc24, ky).
            # (pt,z) merge is exact: rec stride 96 over P*4, w16 stride 4 over
            # P*4 (16 = 4*4).
            # 1) multiply by W16[z,ky] (bcast xs,c) -- bf16 2x
            nc.vector.tensor_tensor(
                _ap(rec[:], 0, [[P * RECE, 128], [96, P * 4], [4, 24], [1, 4]]),
                _ap(rec[:], 0, [[P * RECE, 128], [96, P * 4], [4, 24], [1, 4]]),
                _ap(w16[:], 0, [[P * 16, 128], [4, P * 4], [0, 24], [1, 4]]),
                mybir.AluOpType.mult)
            # 2) tree-reduce z (outer dim; fully packed)
            s192 = prods.tile([128, P * 192], BF16)
            nc.vector.tensor_tensor(
                _ap(s192[:], 0, [[P * 192, 128], [192, P], [96, 2], [1, 96]]),
                _ap(rec[:], 0, [[P * RECE, 128], [RECE, P], [96, 2], [1, 96]]),
                _ap(rec[:], 192, [[P * RECE, 128], [RECE, P], [96, 2], [1, 96]]),
                mybir.AluOpType.add)
            s96 = prods.tile([128, P * 96], BF16)
            nc.vector.tensor_tensor(
                _ap(s96[:], 0, [[P * 96, 128], [96, P], [1, 96]]),
                _ap(s192[:], 0, [[P * 192, 128], [192, P], [1, 96]]),
                _ap(s192[:], 96, [[P * 192, 128], [192, P], [1, 96]]),
                mybir.AluOpType.add)
            # 3) tree-reduce ky: s96[pt, xs8, c3, ky4] -> s24[pt, xs8, c3]
            #    L2 runs on gpsimd (stride-2 input is 1x on DVE anyway) and
            #    promotes to f32 for the remaining accumulation.
            s48 = prods.tile([128, P * 48], BF16)
            nc.vector.tensor_tensor(
                _ap(s48[:], 0, [[P * 48, 128], [48, P], [2, 24], [1, 2]]),
                _ap(s96[:], 0, [[P * 96, 128], [96, P], [4, 24], [1, 2]]),
                _ap(s96[:], 2, [[P * 96, 128], [96, P], [4, 24], [1, 2]]),
                mybir.AluOpType.add)
            s24 = prods.tile([128, P * 24], F32)
            nc.gpsimd.tensor_tensor(
                _ap(s24[:], 0, [[P * 24, 128], [24, P], [1, 24]]),
                _ap(s48[:], 0, [[P * 48, 128], [48, P], [2, 24]]),
                _ap(s48[:], 1, [[P * 48, 128], [48, P], [2, 24]]),
                mybir.AluOpType.add)
            # 4) multiply by W8c[xs,c] (f32 x bf16 -> f32)
            nc.vector.tensor_tensor(
                _ap(s24[:], 0, [[P * 24, 128], [1, P * 24]]),
                _ap(s24[:], 0, [[P * 24, 128], [1, P * 24]]),
                _ap(w8c[:], 0, [[P * 24, 128], [1, P * 24]]),
                mybir.AluOpType.mult)
            # 5) tree-reduce xs in f32: [xs8, c3] -> [c3]
            s12 = touts.tile([128, P * 12], F32)
            nc.vector.tensor_tensor(
                _ap(s12[:], 0, [[P * 12, 128], [12, P], [1, 12]]),
                _ap(s24[:], 0, [[P * 24, 128], [24, P], [1, 12]]),
                _ap(s24[:], 12, [[P * 24, 128], [24, P], [1, 12]]),
                mybir.AluOpType.add)
            s6 = touts.tile([128, P * 6], F32)
            nc.vector.tensor_tensor(
                _ap(s6[:], 0, [[P * 6, 128], [6, P], [1, 6]]),
                _ap(s12[:], 0, [[P * 12, 128], [12, P], [1, 6]]),
                _ap(s12[:], 6, [[P * 12, 128], [12, P], [1, 6]]),
                mybir.AluOpType.add)
            t_c = touts.tile([128, P * 3], F32)
            nc.vector.tensor_tensor(
                _ap(t_c[:], 0, [[P * 3, 128], [3, P], [1, 3]]),
                _ap(s6[:], 0, [[P * 6, 128], [6, P], [1, 3]]),
                _ap(s6[:], 3, [[P * 6, 128], [6, P], [1, 3]]),
                mybir.AluOpType.add)

            nc.sync.dma_start(
                out=t_out[:, ch * P:(ch + 1) * P, :],
                in_=t_c[:].rearrange("p (a b) -> p a b", b=3))


# ======================================================================
# Self-contained entry point: kernel(**inputs) -> np.ndarray
# ======================================================================

N_POINTS = 2_000_000
N_CORES = 8
PTS_PER_CORE = N_POINTS // N_CORES      # 250000
PAD_PER_CORE = 128 * COLS               # 253952

_CACHE = {}


def _build_nc(trace_sim=False, compile_=True):
    import concourse.bacc as bacc

    nc = bacc.Bacc(
        "TRN2",
        target_bir_lowering=False,
        debug=False,
        num_devices=N_CORES,
    )
    xs = nc.dram_tensor("xs", [128, COLS], F32, kind="ExternalInput").ap()
    ys = nc.dram_tensor("ys", [128, COLS], F32, kind="ExternalInput").ap()
    zs = nc.dram_tensor("zs", [128, COLS], F32, kind="ExternalInput").ap()
    phi = nc.dram_tensor("phi", [128, 128 * ZC], BF16, kind="ExternalInput").ap()
    t_out = nc.dram_tensor("t_out", [128, COLS, NC_], F32, kind="ExternalOutput").ap()

    with tile.TileContext(nc, trace_sim=trace_sim) as tc:
        bspline_kernel(tc, [t_out], [xs, ys, zs, phi])
    if compile_:
        nc.compile()
    return nc


def get_nc():
    if "nc" not in _CACHE:
        _CACHE["nc"] = _build_nc()
    return _CACHE["nc"]


def _shard(arr):
    """[N_POINTS] -> list of 8 [128, COLS] arrays (padded with zeros)."""
    out = []
    for c in range(N_CORES):
        s = arr[c * PTS_PER_CORE:(c + 1) * PTS_PER_CORE]
        p = np.zeros(PAD_PER_CORE, dtype=np.float32)
        p[:PTS_PER_CORE] = s
        out.append(p.reshape(128, COLS))
    return out


def _prep_phi(phi_x):
    """[128,128,128,3] f32 x-major -> [y, x*(z*c)] bf16 as uint16 view."""
    import ml_dtypes
    pt = np.ascontiguousarray(phi_x.transpose(1, 0, 2, 3)).reshape(128, 128 * ZC)
    return pt.astype(ml_dtypes.bfloat16).view(np.uint16)


def run_on_cores(x, y, z, phi_x, trace=False, **kw):
    from concourse.bass_utils import run_bass_kernel_spmd

    nc = get_nc()
    xsh, ysh, zsh = _shard(x), _shard(y), _shard(z)
    phi_r = _prep_phi(phi_x)
    in_maps = [
        {"xs": xsh[c], "ys": ysh[c], "zs": zsh[c], "phi": phi_r}
        for c in range(N_CORES)
    ]
    res = run_bass_kernel_spmd(
        nc, in_maps, core_ids=list(range(N_CORES)), trace=trace, **kw
    )
    outs = []
    for c in range(N_CORES):
        t = res.results[c]["t_out"].reshape(PAD_PER_CORE, NC_)
        outs.append(t[:PTS_PER_CORE])
    full = np.concatenate(outs, axis=0).astype(np.float32)
    return full, res


def kernel(x, y, z, phi_x):
    full, _ = run_on_cores(
        np.asarray(x, dtype=np.float32),
        np.asarray(y, dtype=np.float32),
        np.asarray(z, dtype=np.float32),
        np.asarray(phi_x, dtype=np.float32),
    )
    return full


# revision 30
# speedup vs baseline: 1.0909x; 1.0909x over previous
"""Bass/Tile kernel for BSplineField3d (tricubic B-spline interpolation).

v2 design (cost-model-driven):
  Table: Cy8[xq=32, yc=125, z=128, xs=8, c=3, ky=4] in bf16 (98 MB DRAM).
    Cy8[xq,yc,z,xs,c,ky] = sum_m A[ky,m] * phi[min(4*xq+xs,127), yc+m, z, c]
    The y-dimension is pre-contracted into a degree-3 polynomial in v
    (coefficient index ky); the 8-wide x-slot span (xs) makes each point's
    full 64-tap data ONE contiguous 768-byte record:
        rec(xq, yc, z0) = Cy8[xq, yc, z0:z0+4, :, :, :]   (384 bf16)
    built with PE matmuls (bf16) against a banded B-spline matrix.
  Phase 2: per chunk of 128xP points:
    - cell indices + fractional coords (DVE/Act)
    - ONE indirect-DMA record per point (1984 gather instructions total,
      each 128 records of 768B, issued on gpsimd)
    - combine on DVE in bf16 (2x packed mode):
        mult by W16[z,ky] = ww[z]*v^ky, tree-reduce z, tree-reduce ky,
        mult by W8c[xs,c] = w8[xs] (masked wu), tree-reduce xs.
"""

from contextlib import ExitStack

import sys as _sys
for _p in ("/opt/trn_rl_repo",):
    if _p not in _sys.path:
        _sys.path.append(_p)

import numpy as np

import concourse.bass as bass
import concourse.tile as tile
from concourse import mybir
from concourse._compat import with_exitstack

F32 = mybir.dt.float32
BF16 = mybir.dt.bfloat16
I32 = mybir.dt.int32

NX = 128          # grid points per dim
NCELL = 125       # valid cells per dim (ix in [0,124])
NC_ = 3           # components
ZC = NX * NC_     # 384 floats per (y,x) z-row in transposed phi
UNIT = 96         # bf16 elems per (xq,yc,z): [xs8, c3, ky4]
RECE = 4 * UNIT   # 384 elems per record (z-window of 4 units)
NXQ = 32
TAB_ELEMS = NXQ * NCELL * NX * UNIT  # 49,152,000

COLS = 1984       # points per partition (128*1984 = 253952 >= 250000)
P = 64            # points per partition per chunk
NCHUNK = COLS // P  # 31

INV_D = 62.5      # 1/dx, dx = 2/125


def bspline_poly_A():
    """A[k][m]: coefficient of v^k in the cubic B-spline weight of tap m."""
    return np.array(
        [
            [1 / 6, 4 / 6, 1 / 6, 0.0],
            [-3 / 6, 0.0, 3 / 6, 0.0],
            [3 / 6, -6 / 6, 3 / 6, 0.0],
            [-1 / 6, 3 / 6, -3 / 6, 1 / 6],
        ],
        dtype=np.float64,
    )


def build_W_const():
    """W[y, ky*125+yc] = A[ky, y-yc] for 0 <= y-yc <= 3 else 0; bf16 [128, 500]."""
    import ml_dtypes
    A = bspline_poly_A()
    W = np.zeros((128, 4, 125), np.float32)
    for yc in range(NCELL):
        for m in range(4):
            for k in range(4):
                W[yc + m, k, yc] = A[k, m]
    return W.reshape(128, 500).astype(ml_dtypes.bfloat16)


def _ap(t, offset, dims):
    """Raw AP on the same tensor as AP `t` with explicit [step, num] dims."""
    return bass.AP(tensor=t.tensor, offset=t.offset + offset, ap=[list(d) for d in dims])


@with_exitstack
def bspline_kernel(ctx: ExitStack, tc: tile.TileContext, outs, ins):
    """outs = [T_out [128, COLS, 3] f32]
    ins  = [xs, ys, zs [128, COLS] f32, phi_t [128, 49152] bf16 (y-major)]"""
    nc = tc.nc
    xs, ys, zs, phi = ins
    t_out = outs[0]

    w_dram = nc.inline_tensor(build_W_const(), name="w_const")

    dram = ctx.enter_context(tc.tile_pool(name="cydram", bufs=1, space="DRAM"))
    cy = dram.tile([NXQ, NCELL, NX * UNIT], BF16)

    # ---------------- Phase 1: build Cy8 ----------------
    with ExitStack() as p1:
        singles = p1.enter_context(tc.tile_pool(name="p1_singles", bufs=1))
        stages = p1.enter_context(tc.tile_pool(name="p1_stage", bufs=1))
        psums = p1.enter_context(tc.psum_pool(name="p1_psum", bufs=2))

        w_sb = singles.tile([128, 500], BF16)
        nc.sync.dma_start(out=w_sb[:], in_=w_dram.ap())
        phi_sb = singles.tile([128, 128 * ZC], BF16)
        # 4 loads so they pipeline with the first matmuls
        for q in range(4):
            nc.sync.dma_start(
                out=phi_sb[:, q * 32 * ZC:(q + 1) * 32 * ZC],
                in_=_ap(phi, q * 32 * ZC, [[128 * ZC, 128], [1, 32 * ZC]]),
            )

        # per x: one psum [125, 4*512] (4 banks, one per ky); ONE strided copy
        # into stage(xq) slots 0-3.  Slots 4-7 of stage(xq-1) equal slots 0-3
        # of stage(xq), filled by one packed bf16 SBUF block-copy per xq.
        cp_engines = [nc.scalar, nc.vector]
        wr_engines = [nc.sync, nc.sync, nc.scalar, nc.gpsimd]
        stage_tiles = {}

        for x in range(NX):
            ps = psums.tile([NCELL, 4 * 512], F32)
            for ky in range(4):
                nc.tensor.matmul(
                    ps[:, ky * 512:ky * 512 + ZC],
                    w_sb[:, ky * NCELL:(ky + 1) * NCELL],
                    phi_sb[:, x * ZC:(x + 1) * ZC],
                    start=True,
                    stop=True,
                )
            xq_a, s_a = x // 4, x % 4
            if xq_a not in stage_tiles:
                stage_tiles[xq_a] = stages.tile([128, NX * UNIT], BF16,
                                                name=f"stage{xq_a % 3}")
            stage = stage_tiles[xq_a]
            # psum [yc, ky*512 + z*3 + c] f32 -> stage[yc, z*96 + s_a*12 + c*4 + ky]
            src = _ap(ps[:], 0, [[4 * 512, NCELL], [3, NX], [1, NC_], [512, 4]])
            dst = _ap(stage[:], s_a * 12,
                      [[NX * UNIT, NCELL], [UNIT, NX], [4, NC_], [1, 4]])
            eng = cp_engines[x % 2]
            if eng is nc.scalar:
                eng.copy(out=dst, in_=src)
            else:
                eng.tensor_copy(out=dst, in_=src)

            if s_a == 3:
                if xq_a >= 1:
                    # stage(xq-1)[z, slots4-7] = stage(xq)[z, slots0-3]
                    prev = stage_tiles[xq_a - 1]
                    nc.vector.tensor_copy(
                        out=_ap(prev[:], 48, [[NX * UNIT, NCELL], [UNIT, NX], [1, 48]]),
                        in_=_ap(stage[:], 0, [[NX * UNIT, NCELL], [UNIT, NX], [1, 48]]))
                    wr_engines[(xq_a - 1) % len(wr_engines)].dma_start(
                        out=cy[xq_a - 1, :, :], in_=prev[:NCELL, :])
                    del stage_tiles[xq_a - 1]
                if x == NX - 1:
                    # records of xq=31 cover x=124..131; replicate x=127 (slot 3)
                    # into slots 4-7 (masked out by w8, only need finite data)
                    nc.vector.tensor_copy(
                        out=_ap(stage[:], 48, [[NX * UNIT, NCELL], [UNIT, NX], [12, 4], [1, 12]]),
                        in_=_ap(stage[:], 36, [[NX * UNIT, NCELL], [UNIT, NX], [0, 4], [1, 12]]))
                    wr_engines[xq_a % len(wr_engines)].dma_start(
                        out=cy[xq_a, :, :], in_=stage[:NCELL, :])
                    del stage_tiles[xq_a]

    # ---------------- Phase 2: points ----------------
    cy_flat = _ap(cy[:], 0, [[TAB_ELEMS, 1], [1, TAB_ELEMS]])
    ident = nc.inline_tensor(np.eye(128, dtype=np.float32).astype(
        __import__("ml_dtypes").bfloat16), name="ident")

    with ExitStack() as p2:
        sing = p2.enter_context(tc.tile_pool(name="p2_singles", bufs=1))
        coords = p2.enter_context(tc.tile_pool(name="p2_coords", bufs=3))
        small = p2.enter_context(tc.tile_pool(name="p2_small", bufs=2))
        idxp = p2.enter_context(tc.tile_pool(name="p2_idx", bufs=3))
        recs = p2.enter_context(tc.tile_pool(name="p2_rec", bufs=2))
        prods = p2.enter_context(tc.tile_pool(name="p2_prod", bufs=1))
        touts = p2.enter_context(tc.tile_pool(name="p2_tout", bufs=2))
        psums2 = p2.enter_context(tc.psum_pool(name="p2_psum", bufs=1))

        # j-ramp constant: [128, 8] = 0..7 (x-slot index within record)
        jr8f = sing.tile([128, 8], F32)
        nc.gpsimd.iota(jr8f[:], [[1, 8]], channel_multiplier=0,
                       allow_small_or_imprecise_dtypes=True)
        jr8 = sing.tile([128, 8], BF16)
        nc.scalar.copy(out=jr8[:], in_=jr8f[:])
        id_sb = sing.tile([128, 128], BF16)
        nc.sync.dma_start(out=id_sb[:], in_=ident.ap())

        for ch in range(NCHUNK):
            x_t = coords.tile([128, P], F32)
            y_t = coords.tile([128, P], F32)
            z_t = coords.tile([128, P], F32)
            nc.sync.dma_start(out=x_t[:], in_=xs[:, ch * P:(ch + 1) * P])
            nc.sync.dma_start(out=y_t[:], in_=ys[:, ch * P:(ch + 1) * P])
            nc.sync.dma_start(out=z_t[:], in_=zs[:, ch * P:(ch + 1) * P])

            # --- cell indices + fractions ---
            def exact_floor(src, out, sfx):
                # out = floor(src) for src >= 0; f32<->i32 converts round to
                # nearest, so correct with an is_gt mask. Converts on Act.
                ci = small.tile([128, P], I32, name=f"ci_{sfx}")
                cf = small.tile([128, P], F32, name=f"cf_{sfx}")
                nc.scalar.copy(out=ci[:], in_=src[:])
                nc.scalar.copy(out=cf[:], in_=ci[:])
                nc.vector.tensor_tensor(out[:], cf[:], src[:], mybir.AluOpType.is_gt)
                nc.vector.tensor_sub(out[:], cf[:], out[:])

            def split_coord(src, sfx):
                u = small.tile([128, P], F32, name=f"u_{sfx}")
                fr = small.tile([128, P], F32, name=f"fr_{sfx}")
                ii = small.tile([128, P], F32, name=f"ii_{sfx}")
                # u = (x+1)*62.5 on Act
                nc.scalar.activation(u[:], src[:],
                                     mybir.ActivationFunctionType.Copy,
                                     bias=INV_D, scale=INV_D)
                exact_floor(u, ii, sfx)
                nc.vector.tensor_scalar(ii[:], ii[:], float(NCELL - 1), 0.0,
                                        mybir.AluOpType.min, mybir.AluOpType.max)
                nc.vector.tensor_sub(fr[:], u[:], ii[:])
                return ii, fr

            ix_t, fu = split_coord(x_t, "x")
            iy_t, fv = split_coord(y_t, "y")
            iz_t, fw = split_coord(z_t, "z")

            # --- x-quad decomposition: xq = ix//4, s0 = ix%4 ---
            tq = small.tile([128, P], F32)
            xq_t = small.tile([128, P], F32)
            s0_t = small.tile([128, P], F32)
            nc.vector.tensor_scalar(tq[:], ix_t[:], 0.25, None, mybir.AluOpType.mult)
            exact_floor(tq, xq_t, "q")
            nc.vector.scalar_tensor_tensor(
                s0_t[:], xq_t[:], -4.0, ix_t[:],
                mybir.AluOpType.mult, mybir.AluOpType.add)

            # --- record index (elem units): ((xq*125+yc)*128+z0)*96
            #     = 32 * (3 * (xq*16000 + yc*128 + z0))
            byz = small.tile([128, P], F32)
            nc.vector.scalar_tensor_tensor(
                byz[:], iy_t[:], float(NX), iz_t[:],
                mybir.AluOpType.mult, mybir.AluOpType.add)
            idx_f = small.tile([128, P], F32)
            nc.vector.scalar_tensor_tensor(
                idx_f[:], xq_t[:], 16000.0, byz[:],
                mybir.AluOpType.mult, mybir.AluOpType.add)
            nc.vector.tensor_scalar(idx_f[:], idx_f[:], 3.0, None,
                                    mybir.AluOpType.mult)
            idx_i = idxp.tile([128, P], I32)
            nc.vector.tensor_copy(out=idx_i[:], in_=idx_f[:])
            nc.vector.tensor_scalar(idx_i[:], idx_i[:], 32, None,
                                    mybir.AluOpType.mult)

            # --- tap weights ---
            def tap_weights(fr, sfx):
                wt = small.tile([128, P, 4], F32, name=f"wt_{sfx}")
                t = small.tile([128, P], F32, name=f"t_{sfx}")
                t2 = small.tile([128, P], F32, name=f"t2_{sfx}")
                r2 = small.tile([128, P], F32, name=f"r2_{sfx}")
                r3 = small.tile([128, P], F32, name=f"r3_{sfx}")
                w0 = wt[:, :, 0]
                w1 = wt[:, :, 1]
                w2 = wt[:, :, 2]
                w3 = wt[:, :, 3]
                nc.vector.tensor_scalar(t[:], fr[:], -1.0, 1.0,
                                        mybir.AluOpType.mult, mybir.AluOpType.add)
                nc.scalar.square(t2[:], t[:])
                nc.vector.scalar_tensor_tensor(w0, t2[:], 1 / 6, t[:],
                                               mybir.AluOpType.mult, mybir.AluOpType.mult)
                nc.scalar.square(r2[:], fr[:])
                nc.vector.tensor_mul(r3[:], r2[:], fr[:])
                nc.vector.tensor_scalar(w3, r3[:], 1 / 6, None, mybir.AluOpType.mult)
                nc.vector.scalar_tensor_tensor(w1, r3[:], 0.5, r2[:],
                                               mybir.AluOpType.mult, mybir.AluOpType.subtract)
                nc.vector.tensor_scalar(w1, w1, 2 / 3, None, mybir.AluOpType.add)
                nc.vector.tensor_add(w2, w0, w1)
                nc.vector.tensor_add(w2, w2, w3)
                nc.vector.tensor_scalar(w2, w2, -1.0, 1.0,
                                        mybir.AluOpType.mult, mybir.AluOpType.add)
                return wt

            wu = tap_weights(fu, "u")
            ww = tap_weights(fw, "w")

            vp = small.tile([128, P, 4], F32)
            nc.vector.memset(vp[:, :, 0], 1.0)
            nc.vector.tensor_copy(out=vp[:, :, 1], in_=fv[:])
            nc.scalar.square(vp[:, :, 2], fv[:])
            nc.vector.tensor_mul(vp[:, :, 3], vp[:, :, 2], fv[:])

            # --- W16[pt, z4, ky4] = ww[z] * v^ky  (bf16) ---
            w16 = small.tile([128, P, 16], BF16)
            nc.vector.tensor_tensor(
                _ap(w16[:], 0, [[P * 16, 128], [16, P], [4, 4], [1, 4]]),
                _ap(ww[:], 0, [[P * 4, 128], [4, P], [1, 4], [0, 4]]),
                _ap(vp[:], 0, [[P * 4, 128], [4, P], [0, 4], [1, 4]]),
                mybir.AluOpType.mult)

            # --- w8[pt, j] = wu[j - s0] for j-s0 in [0,4) else 0 ---
            # whole chain in bf16 (d8/e8 are small ints / 0-1 masks, exact in
            # bf16; wu is rounded to bf16 exactly once, same as v2's W8c)
            s0b = small.tile([128, P], BF16)
            wub = small.tile([128, P, 4], BF16)
            nc.scalar.copy(out=s0b[:], in_=s0_t[:])
            nc.scalar.copy(out=wub[:], in_=wu[:])
            d8 = small.tile([128, P, 8], BF16)
            e8 = small.tile([128, P, 8], BF16)
            w8 = small.tile([128, P, 8], BF16)
            nc.vector.tensor_tensor(
                _ap(d8[:], 0, [[P * 8, 128], [8, P], [1, 8]]),
                _ap(jr8[:], 0, [[8, 128], [0, P], [1, 8]]),
                _ap(s0b[:], 0, [[P, 128], [1, P], [0, 8]]),
                mybir.AluOpType.subtract)
            for l in range(4):
                tgt = w8 if l == 0 else e8
                nc.vector.tensor_scalar(e8[:], d8[:], float(l), None,
                                        mybir.AluOpType.is_equal)
                nc.vector.tensor_tensor(
                    _ap(tgt[:], 0, [[P * 8, 128], [8, P], [1, 8]]),
                    _ap(e8[:], 0, [[P * 8, 128], [8, P], [1, 8]]),
                    _ap(wub[:], l, [[P * 4, 128], [4, P], [0, 8]]),
                    mybir.AluOpType.mult)
                if l > 0:
                    nc.vector.tensor_add(w8[:], w8[:], e8[:])

            # --- W8c[pt, xs8, c3] = w8[xs] replicated over c (bf16, Act) ---
            w8c = small.tile([128, P, 24], BF16)
            nc.scalar.copy(
                out=_ap(w8c[:], 0, [[P * 24, 128], [24, P], [3, 8], [1, 3]]),
                in_=_ap(w8[:], 0, [[P * 8, 128], [8, P], [1, 8], [0, 3]]))

            # --- gather: one 768B record per point ---
            rec = recs.tile([128, P * RECE], BF16)
            for t in range(P):
                nc.gpsimd.indirect_dma_start(
                    out=_ap(rec[:], t * RECE, [[P * RECE, 128], [1, RECE]]),
                    out_offset=None,
                    in_=cy_flat,
                    in_offset=bass.IndirectOffsetOnAxis(
                        ap=_ap(idx_i[:], t, [[P, 128], [1, 1]]), axis=1),
                )

            # --- combine ---
            # rec[pt, z4, xs8, c3, ky4]; iteration ((pt,z) merged, xsc24, ky).
            # (pt,z) merge is exact: rec stride 96 over P*4, w16 stride 4 over
            # P*4 (16 = 4*4).
            # 1) multiply by W16[z,ky] (bcast xs,c) -- bf16 2x
            nc.vector.tensor_tensor(
                _ap(rec[:], 0, [[P * RECE, 128], [96, P * 4], [4, 24], [1, 4]]),
                _ap(rec[:], 0, [[P * RECE, 128], [96, P * 4], [4, 24], [1, 4]]),
                _ap(w16[:], 0, [[P * 16, 128], [4, P * 4], [0, 24], [1, 4]]),
                mybir.AluOpType.mult)
            # 2) tree-reduce z (outer dim; fully packed)
            s192 = prods.tile([128, P * 192], BF16)
            nc.vector.tensor_tensor(
                _ap(s192[:], 0, [[P * 192, 128], [192, P], [96, 2], [1, 96]]),
                _ap(rec[:], 0, [[P * RECE, 128], [RECE, P], [96, 2], [1, 96]]),
                _ap(rec[:], 192, [[P * RECE, 128], [RECE, P], [96, 2], [1, 96]]),
                mybir.AluOpType.add)
            # tree-z L2 on PE: psum = I*s192_lo + I*s192_hi (f32 accumulate),
            # then Act drains psum -> s96 bf16.  4-point pieces (384 cols,
            # under the 512-col matmul ISA limit), 4 rotating psum banks.
            s96 = prods.tile([128, P * 96], BF16)
            NPC = 4           # points per piece
            for pc in range(P // NPC):
                psz = psums2.tile([128, NPC * 96], F32, name=f"psz{pc % 4}")
                nc.tensor.matmul(
                    psz[:],
                    id_sb[:],
                    _ap(s192[:], pc * NPC * 192,
                        [[P * 192, 128], [192, NPC], [1, 96]]),
                    start=True, stop=False)
                nc.tensor.matmul(
                    psz[:],
                    id_sb[:],
                    _ap(s192[:], pc * NPC * 192 + 96,
                        [[P * 192, 128], [192, NPC], [1, 96]]),
                    start=False, stop=True)
                nc.scalar.copy(
                    out=_ap(s96[:], pc * NPC * 96, [[P * 96, 128], [1, NPC * 96]]),
                    in_=psz[:])
            # 3) tree-reduce ky: s96[pt, xs8, c3, ky4] -> s24[pt, xs8, c3]
            #    L2 runs on gpsimd (stride-2 input is 1x on DVE anyway) and
            #    promotes to f32 for the remaining accumulation.
            s48 = prods.tile([128, P * 48], BF16)
            nc.vector.tensor_tensor(
                _ap(s48[:], 0, [[P * 48, 128], [48, P], [2, 24], [1, 2]]),
                _ap(s96[:], 0, [[P * 96, 128], [96, P], [4, 24], [1, 2]]),
                _ap(s96[:], 2, [[P * 96, 128], [96, P], [4, 24], [1, 2]]),
                mybir.AluOpType.add)
            s24 = prods.tile([128, P * 24], F32)
            nc.gpsimd.tensor_tensor(
                _ap(s24[:], 0, [[P * 24, 128], [24, P], [1, 24]]),
                _ap(s48[:], 0, [[P * 48, 128], [48, P], [2, 24]]),
                _ap(s48[:], 1, [[P * 48, 128], [48, P], [2, 24]]),
                mybir.AluOpType.add)
            # 4) multiply by W8c[xs,c] (f32 x bf16 -> f32)
            nc.vector.tensor_tensor(
                _ap(s24[:], 0, [[P * 24, 128], [1, P * 24]]),
                _ap(s24[:], 0, [[P * 24, 128], [1, P * 24]]),
                _ap(w8c[:], 0, [[P * 24, 128], [1, P * 24]]),
                mybir.AluOpType.mult)
            # 5) tree-reduce xs in f32: [xs8, c3] -> [c3]
            s12 = touts.tile([128, P * 12], F32)
            nc.vector.tensor_tensor(
                _ap(s12[:], 0, [[P * 12, 128], [12, P], [1, 12]]),
                _ap(s24[:], 0, [[P * 24, 128], [24, P], [1, 12]]),
                _ap(s24[:], 12, [[P * 24, 128], [24, P], [1, 12]]),
                mybir.AluOpType.add)
            s6 = touts.tile([128, P * 6], F32)
            nc.vector.tensor_tensor(
                _ap(s6[:], 0, [[P * 6, 128], [6, P], [1, 6]]),
                _ap(s12[:], 0, [[P * 12, 128], [12, P], [1, 6]]),
                _ap(s12[:], 6, [[P * 12, 128], [12, P], [1, 6]]),
                mybir.AluOpType.add)
            t_c = touts.tile([128, P * 3], F32)
            nc.vector.tensor_tensor(
                _ap(t_c[:], 0, [[P * 3, 128], [3, P], [1, 3]]),
                _ap(s6[:], 0, [[P * 6, 128], [6, P], [1, 3]]),
                _ap(s6[:], 3, [[P * 6, 128], [6, P], [1, 3]]),
                mybir.AluOpType.add)

            nc.sync.dma_start(
                out=t_out[:, ch * P:(ch + 1) * P, :],
                in_=t_c[:].rearrange("p (a b) -> p a b", b=3))


# ======================================================================
# Self-contained entry point: kernel(**inputs) -> np.ndarray
# ======================================================================

N_POINTS = 2_000_000
N_CORES = 8
PTS_PER_CORE = N_POINTS // N_CORES      # 250000
PAD_PER_CORE = 128 * COLS               # 253952

_CACHE = {}


def _build_nc(trace_sim=False, compile_=True):
    import concourse.bacc as bacc

    nc = bacc.Bacc(
        "TRN2",
        target_bir_lowering=False,
        debug=False,
        num_devices=N_CORES,
    )
    xs = nc.dram_tensor("xs", [128, COLS], F32, kind="ExternalInput").ap()
    ys = nc.dram_tensor("ys", [128, COLS], F32, kind="ExternalInput").ap()
    zs = nc.dram_tensor("zs", [128, COLS], F32, kind="ExternalInput").ap()
    phi = nc.dram_tensor("phi", [128, 128 * ZC], BF16, kind="ExternalInput").ap()
    t_out = nc.dram_tensor("t_out", [128, COLS, NC_], F32, kind="ExternalOutput").ap()

    with tile.TileContext(nc, trace_sim=trace_sim) as tc:
        bspline_kernel(tc, [t_out], [xs, ys, zs, phi])
    if compile_:
        nc.compile()
    return nc


def get_nc():
    if "nc" not in _CACHE:
        _CACHE["nc"] = _build_nc()
    return _CACHE["nc"]


def _shard(arr):
    """[N_POINTS] -> list of 8 [128, COLS] arrays (padded with zeros)."""
    out = []
    for c in range(N_CORES):
        s = arr[c * PTS_PER_CORE:(c + 1) * PTS_PER_CORE]
        p = np.zeros(PAD_PER_CORE, dtype=np.float32)
        p[:PTS_PER_CORE] = s
        out.append(p.reshape(128, COLS))
    return out


def _prep_phi(phi_x):
    """[128,128,128,3] f32 x-major -> [y, x*(z*c)] bf16 as uint16 view."""
    import ml_dtypes
    pt = np.ascontiguousarray(phi_x.transpose(1, 0, 2, 3)).reshape(128, 128 * ZC)
    return pt.astype(ml_dtypes.bfloat16).view(np.uint16)


def run_on_cores(x, y, z, phi_x, trace=False, **kw):
    from concourse.bass_utils import run_bass_kernel_spmd

    nc = get_nc()
    xsh, ysh, zsh = _shard(x), _shard(y), _shard(z)
    phi_r = _prep_phi(phi_x)
    in_maps = [
        {"xs": xsh[c], "ys": ysh[c], "zs": zsh[c], "phi": phi_r}
        for c in range(N_CORES)
    ]
    res = run_bass_kernel_spmd(
        nc, in_maps, core_ids=list(range(N_CORES)), trace=trace, **kw
    )
    outs = []
    for c in range(N_CORES):
        t = res.results[c]["t_out"].reshape(PAD_PER_CORE, NC_)
        outs.append(t[:PTS_PER_CORE])
    full = np.concatenate(outs, axis=0).astype(np.float32)
    return full, res


def kernel(x, y, z, phi_x):
    full, _ = run_on_cores(
        np.asarray(x, dtype=np.float32),
        np.asarray(y, dtype=np.float32),
        np.asarray(z, dtype=np.float32),
        np.asarray(phi_x, dtype=np.float32),
    )
    return full


# revision 32
# speedup vs baseline: 1.1067x; 1.0145x over previous
"""Bass/Tile kernel for BSplineField3d (tricubic B-spline interpolation).

v2 design (cost-model-driven):
  Table: Cy8[xq=32, yc=125, z=128, xs=8, c=3, ky=4] in bf16 (98 MB DRAM).
    Cy8[xq,yc,z,xs,c,ky] = sum_m A[ky,m] * phi[min(4*xq+xs,127), yc+m, z, c]
    The y-dimension is pre-contracted into a degree-3 polynomial in v
    (coefficient index ky); the 8-wide x-slot span (xs) makes each point's
    full 64-tap data ONE contiguous 768-byte record:
        rec(xq, yc, z0) = Cy8[xq, yc, z0:z0+4, :, :, :]   (384 bf16)
    built with PE matmuls (bf16) against a banded B-spline matrix.
  Phase 2: per chunk of 128xP points:
    - cell indices + fractional coords (DVE/Act)
    - ONE indirect-DMA record per point (1984 gather instructions total,
      each 128 records of 768B, issued on gpsimd)
    - combine on DVE in bf16 (2x packed mode):
        mult by W16[z,ky] = ww[z]*v^ky, tree-reduce z, tree-reduce ky,
        mult by W8c[xs,c] = w8[xs] (masked wu), tree-reduce xs.
"""

from contextlib import ExitStack

import sys as _sys
for _p in ("/opt/trn_rl_repo",):
    if _p not in _sys.path:
        _sys.path.append(_p)

import numpy as np

import concourse.bass as bass
import concourse.tile as tile
from concourse import mybir
from concourse._compat import with_exitstack

F32 = mybir.dt.float32
BF16 = mybir.dt.bfloat16
I32 = mybir.dt.int32

NX = 128          # grid points per dim
NCELL = 125       # valid cells per dim (ix in [0,124])
NC_ = 3           # components
ZC = NX * NC_     # 384 floats per (y,x) z-row in transposed phi
UNIT = 96         # bf16 elems per (xq,yc,z): [xs8, c3, ky4]
RECE = 4 * UNIT   # 384 elems per record (z-window of 4 units)
NXQ = 32
TAB_ELEMS = NXQ * NCELL * NX * UNIT  # 49,152,000

COLS = 1984       # points per partition (128*1984 = 253952 >= 250000)
P = 64            # points per partition per chunk
NCHUNK = COLS // P  # 31

INV_D = 62.5      # 1/dx, dx = 2/125


def bspline_poly_A():
    """A[k][m]: coefficient of v^k in the cubic B-spline weight of tap m."""
    return np.array(
        [
            [1 / 6, 4 / 6, 1 / 6, 0.0],
            [-3 / 6, 0.0, 3 / 6, 0.0],
            [3 / 6, -6 / 6, 3 / 6, 0.0],
            [-1 / 6, 3 / 6, -3 / 6, 1 / 6],
        ],
        dtype=np.float64,
    )


def build_W_const():
    """W[y, ky*125+yc] = A[ky, y-yc] for 0 <= y-yc <= 3 else 0; bf16 [128, 500]."""
    import ml_dtypes
    A = bspline_poly_A()
    W = np.zeros((128, 4, 125), np.float32)
    for yc in range(NCELL):
        for m in range(4):
            for k in range(4):
                W[yc + m, k, yc] = A[k, m]
    return W.reshape(128, 500).astype(ml_dtypes.bfloat16)


def _ap(t, offset, dims):
    """Raw AP on the same tensor as AP `t` with explicit [step, num] dims."""
    return bass.AP(tensor=t.tensor, offset=t.offset + offset, ap=[list(d) for d in dims])


@with_exitstack
def bspline_kernel(ctx: ExitStack, tc: tile.TileContext, outs, ins):
    """outs = [T_out [128, COLS, 3] f32]
    ins  = [xs, ys, zs [128, COLS] f32, phi_t [128, 49152] bf16 (y-major)]"""
    nc = tc.nc
    xs, ys, zs, phi = ins
    t_out = outs[0]

    w_dram = nc.inline_tensor(build_W_const(), name="w_const")

    dram = ctx.enter_context(tc.tile_pool(name="cydram", bufs=1, space="DRAM"))
    cy = dram.tile([NXQ, NCELL, NX * UNIT], BF16)

    # ---------------- Phase 1: build Cy8 ----------------
    with ExitStack() as p1:
        singles = p1.enter_context(tc.tile_pool(name="p1_singles", bufs=1))
        stages = p1.enter_context(tc.tile_pool(name="p1_stage", bufs=1))
        psums = p1.enter_context(tc.psum_pool(name="p1_psum", bufs=2))

        w_sb = singles.tile([128, 500], BF16)
        nc.sync.dma_start(out=w_sb[:], in_=w_dram.ap())
        phi_sb = singles.tile([128, 128 * ZC], BF16)
        # 4 loads so they pipeline with the first matmuls
        for q in range(4):
            nc.sync.dma_start(
                out=phi_sb[:, q * 32 * ZC:(q + 1) * 32 * ZC],
                in_=_ap(phi, q * 32 * ZC, [[128 * ZC, 128], [1, 32 * ZC]]),
            )

        # per x: one psum [125, 4*512] (4 banks, one per ky); ONE strided copy
        # into stage(xq) slots 0-3.  Slots 4-7 of stage(xq-1) equal slots 0-3
        # of stage(xq), filled by one packed bf16 SBUF block-copy per xq.
        cp_engines = [nc.scalar, nc.vector]
        wr_engines = [nc.sync, nc.sync, nc.scalar, nc.gpsimd]
        stage_tiles = {}

        for x in range(NX):
            ps = psums.tile([NCELL, 4 * 512], F32)
            for ky in range(4):
                nc.tensor.matmul(
                    ps[:, ky * 512:ky * 512 + ZC],
                    w_sb[:, ky * NCELL:(ky + 1) * NCELL],
                    phi_sb[:, x * ZC:(x + 1) * ZC],
                    start=True,
                    stop=True,
                )
            xq_a, s_a = x // 4, x % 4
            if xq_a not in stage_tiles:
                stage_tiles[xq_a] = stages.tile([128, NX * UNIT], BF16,
                                                name=f"stage{xq_a % 3}")
            stage = stage_tiles[xq_a]
            # psum [yc, ky*512 + z*3 + c] f32 -> stage[yc, z*96 + s_a*12 + c*4 + ky]
            src = _ap(ps[:], 0, [[4 * 512, NCELL], [3, NX], [1, NC_], [512, 4]])
            dst = _ap(stage[:], s_a * 12,
                      [[NX * UNIT, NCELL], [UNIT, NX], [4, NC_], [1, 4]])
            eng = cp_engines[x % 2]
            if eng is nc.scalar:
                eng.copy(out=dst, in_=src)
            else:
                eng.tensor_copy(out=dst, in_=src)

            if s_a == 3:
                if xq_a >= 1:
                    # stage(xq-1)[z, slots4-7] = stage(xq)[z, slots0-3]
                    prev = stage_tiles[xq_a - 1]
                    nc.vector.tensor_copy(
                        out=_ap(prev[:], 48, [[NX * UNIT, NCELL], [UNIT, NX], [1, 48]]),
                        in_=_ap(stage[:], 0, [[NX * UNIT, NCELL], [UNIT, NX], [1, 48]]))
                    wr_engines[(xq_a - 1) % len(wr_engines)].dma_start(
                        out=cy[xq_a - 1, :, :], in_=prev[:NCELL, :])
                    del stage_tiles[xq_a - 1]
                if x == NX - 1:
                    # records of xq=31 cover x=124..131; replicate x=127 (slot 3)
                    # into slots 4-7 (masked out by w8, only need finite data)
                    nc.vector.tensor_copy(
                        out=_ap(stage[:], 48, [[NX * UNIT, NCELL], [UNIT, NX], [12, 4], [1, 12]]),
                        in_=_ap(stage[:], 36, [[NX * UNIT, NCELL], [UNIT, NX], [0, 4], [1, 12]]))
                    wr_engines[xq_a % len(wr_engines)].dma_start(
                        out=cy[xq_a, :, :], in_=stage[:NCELL, :])
                    del stage_tiles[xq_a]

    # ---------------- Phase 2: points ----------------
    cy_flat = _ap(cy[:], 0, [[TAB_ELEMS, 1], [1, TAB_ELEMS]])
    ident = nc.inline_tensor(np.eye(128, dtype=np.float32).astype(
        __import__("ml_dtypes").bfloat16), name="ident")

    with ExitStack() as p2:
        sing = p2.enter_context(tc.tile_pool(name="p2_singles", bufs=1))
        coords = p2.enter_context(tc.tile_pool(name="p2_coords", bufs=3))
        small = p2.enter_context(tc.tile_pool(name="p2_small", bufs=2))
        idxp = p2.enter_context(tc.tile_pool(name="p2_idx", bufs=3))
        recs = p2.enter_context(tc.tile_pool(name="p2_rec", bufs=2))
        prods = p2.enter_context(tc.tile_pool(name="p2_prod", bufs=1))
        touts = p2.enter_context(tc.tile_pool(name="p2_tout", bufs=2))
        psums2 = p2.enter_context(tc.psum_pool(name="p2_psum", bufs=1))

        # j-ramp constant: [128, 8] = 0..7 (x-slot index within record)
        jr8f = sing.tile([128, 8], F32)
        nc.gpsimd.iota(jr8f[:], [[1, 8]], channel_multiplier=0,
                       allow_small_or_imprecise_dtypes=True)
        jr8 = sing.tile([128, 8], BF16)
        nc.scalar.copy(out=jr8[:], in_=jr8f[:])
        id_sb = sing.tile([128, 128], BF16)
        nc.sync.dma_start(out=id_sb[:], in_=ident.ap())

        for ch in range(NCHUNK):
            x_t = coords.tile([128, P], F32)
            y_t = coords.tile([128, P], F32)
            z_t = coords.tile([128, P], F32)
            nc.sync.dma_start(out=x_t[:], in_=xs[:, ch * P:(ch + 1) * P])
            nc.sync.dma_start(out=y_t[:], in_=ys[:, ch * P:(ch + 1) * P])
            nc.sync.dma_start(out=z_t[:], in_=zs[:, ch * P:(ch + 1) * P])

            # --- cell indices + fractions ---
            def exact_floor(src, out, sfx):
                # out = floor(src) for src >= 0; f32<->i32 converts round to
                # nearest, so correct with an is_gt mask. Converts on Act.
                n = 3 * P if sfx == "u3" else P
                ci = small.tile([128, n], I32, name=f"ci_{sfx}")
                cf = small.tile([128, n], F32, name=f"cf_{sfx}")
                nc.scalar.copy(out=ci[:], in_=src[:])
                nc.scalar.copy(out=cf[:], in_=ci[:])
                nc.vector.tensor_tensor(out[:], cf[:], src[:], mybir.AluOpType.is_gt)
                nc.vector.tensor_sub(out[:], cf[:], out[:])

            # stacked [128, 3*P] pipeline: one floor/clamp/frac chain for all
            # three coordinates
            u3 = small.tile([128, 3 * P], F32)
            ii3 = small.tile([128, 3 * P], F32)
            fr3 = small.tile([128, 3 * P], F32)
            for j, src in enumerate((x_t, y_t, z_t)):
                nc.scalar.activation(u3[:, j * P:(j + 1) * P], src[:],
                                     mybir.ActivationFunctionType.Copy,
                                     bias=INV_D, scale=INV_D)
            exact_floor(u3, ii3, "u3")
            nc.vector.tensor_scalar(ii3[:], ii3[:], float(NCELL - 1), 0.0,
                                    mybir.AluOpType.min, mybir.AluOpType.max)
            nc.vector.tensor_sub(fr3[:], u3[:], ii3[:])
            ix_t, iy_t, iz_t = (_ap(ii3[:], j * P, [[3 * P, 128], [1, P]])
                                for j in range(3))
            fu, fv, fw = (_ap(fr3[:], j * P, [[3 * P, 128], [1, P]])
                          for j in range(3))

            # --- x-quad decomposition: xq = ix//4, s0 = ix%4 ---
            tq = small.tile([128, P], F32)
            xq_t = small.tile([128, P], F32)
            s0_t = small.tile([128, P], F32)
            nc.vector.tensor_scalar(tq[:], ix_t, 0.25, None, mybir.AluOpType.mult)
            exact_floor(tq, xq_t, "q")
            nc.vector.scalar_tensor_tensor(
                s0_t[:], xq_t[:], -4.0, ix_t,
                mybir.AluOpType.mult, mybir.AluOpType.add)

            # --- record index (elem units): ((xq*125+yc)*128+z0)*96
            #     = 32 * (3 * (xq*16000 + yc*128 + z0))
            byz = small.tile([128, P], F32)
            nc.vector.scalar_tensor_tensor(
                byz[:], iy_t, float(NX), iz_t,
                mybir.AluOpType.mult, mybir.AluOpType.add)
            idx_f = small.tile([128, P], F32)
            nc.vector.scalar_tensor_tensor(
                idx_f[:], xq_t[:], 16000.0, byz[:],
                mybir.AluOpType.mult, mybir.AluOpType.add)
            nc.vector.tensor_scalar(idx_f[:], idx_f[:], 3.0, None,
                                    mybir.AluOpType.mult)
            idx_i = idxp.tile([128, P], I32)
            nc.vector.tensor_copy(out=idx_i[:], in_=idx_f[:])
            nc.vector.tensor_scalar(idx_i[:], idx_i[:], 32, None,
                                    mybir.AluOpType.mult)

            # --- tap weights ---
            def tap_weights(fr, sfx):
                wt = small.tile([128, P, 4], F32, name=f"wt_{sfx}")
                t = small.tile([128, P], F32, name=f"t_{sfx}")
                t2 = small.tile([128, P], F32, name=f"t2_{sfx}")
                r2 = small.tile([128, P], F32, name=f"r2_{sfx}")
                r3 = small.tile([128, P], F32, name=f"r3_{sfx}")
                w0 = wt[:, :, 0]
                w1 = wt[:, :, 1]
                w2 = wt[:, :, 2]
                w3 = wt[:, :, 3]
                nc.vector.tensor_scalar(t[:], fr, -1.0, 1.0,
                                        mybir.AluOpType.mult, mybir.AluOpType.add)
                nc.scalar.square(t2[:], t[:])
                nc.vector.scalar_tensor_tensor(w0, t2[:], 1 / 6, t[:],
                                               mybir.AluOpType.mult, mybir.AluOpType.mult)
                nc.scalar.square(r2[:], fr)
                nc.vector.tensor_mul(r3[:], r2[:], fr)
                nc.vector.tensor_scalar(w3, r3[:], 1 / 6, None, mybir.AluOpType.mult)
                nc.vector.scalar_tensor_tensor(w1, r3[:], 0.5, r2[:],
                                               mybir.AluOpType.mult, mybir.AluOpType.subtract)
                nc.vector.tensor_scalar(w1, w1, 2 / 3, None, mybir.AluOpType.add)
                nc.vector.tensor_add(w2, w0, w1)
                nc.vector.tensor_add(w2, w2, w3)
                nc.vector.tensor_scalar(w2, w2, -1.0, 1.0,
                                        mybir.AluOpType.mult, mybir.AluOpType.add)
                return wt

            wu = tap_weights(fu, "u")
            ww = tap_weights(fw, "w")

            vp = small.tile([128, P, 4], F32)
            nc.vector.memset(vp[:, :, 0], 1.0)
            nc.vector.tensor_copy(out=vp[:, :, 1], in_=fv)
            nc.scalar.square(vp[:, :, 2], fv)
            nc.vector.tensor_mul(vp[:, :, 3], vp[:, :, 2], fv)

            # --- W16[pt, z4, ky4] = ww[z] * v^ky  (bf16) ---
            w16 = small.tile([128, P, 16], BF16)
            nc.gpsimd.tensor_tensor(
                _ap(w16[:], 0, [[P * 16, 128], [16, P], [4, 4], [1, 4]]),
                _ap(ww[:], 0, [[P * 4, 128], [4, P], [1, 4], [0, 4]]),
                _ap(vp[:], 0, [[P * 4, 128], [4, P], [0, 4], [1, 4]]),
                mybir.AluOpType.mult)

            # --- w8[pt, j] = wu[j - s0] for j-s0 in [0,4) else 0 ---
            # whole chain in bf16 (d8/e8 are small ints / 0-1 masks, exact in
            # bf16; wu is rounded to bf16 exactly once, same as v2's W8c)
            s0b = small.tile([128, P], BF16)
            wub = small.tile([128, P, 4], BF16)
            nc.scalar.copy(out=s0b[:], in_=s0_t[:])
            nc.scalar.copy(out=wub[:], in_=wu[:])
            d8 = small.tile([128, P, 8], BF16)
            e8 = small.tile([128, P, 8], BF16)
            w8 = small.tile([128, P, 8], BF16)
            nc.vector.tensor_tensor(
                _ap(d8[:], 0, [[P * 8, 128], [8, P], [1, 8]]),
                _ap(jr8[:], 0, [[8, 128], [0, P], [1, 8]]),
                _ap(s0b[:], 0, [[P, 128], [1, P], [0, 8]]),
                mybir.AluOpType.subtract)
            for l in range(4):
                tgt = w8 if l == 0 else e8
                nc.vector.tensor_scalar(e8[:], d8[:], float(l), None,
                                        mybir.AluOpType.is_equal)
                nc.vector.tensor_tensor(
                    _ap(tgt[:], 0, [[P * 8, 128], [8, P], [1, 8]]),
                    _ap(e8[:], 0, [[P * 8, 128], [8, P], [1, 8]]),
                    _ap(wub[:], l, [[P * 4, 128], [4, P], [0, 8]]),
                    mybir.AluOpType.mult)
                if l > 0:
                    nc.vector.tensor_add(w8[:], w8[:], e8[:])

            # --- W8c[pt, xs8, c3] = w8[xs] replicated over c (bf16, Act) ---
            w8c = small.tile([128, P, 24], BF16)
            nc.scalar.copy(
                out=_ap(w8c[:], 0, [[P * 24, 128], [24, P], [3, 8], [1, 3]]),
                in_=_ap(w8[:], 0, [[P * 8, 128], [8, P], [1, 8], [0, 3]]))

            # --- gather: one 768B record per point ---
            rec = recs.tile([128, P * RECE], BF16)
            for t in range(P):
                nc.gpsimd.indirect_dma_start(
                    out=_ap(rec[:], t * RECE, [[P * RECE, 128], [1, RECE]]),
                    out_offset=None,
                    in_=cy_flat,
                    in_offset=bass.IndirectOffsetOnAxis(
                        ap=_ap(idx_i[:], t, [[P, 128], [1, 1]]), axis=1),
                )

            # --- combine ---
            # rec[pt, z4, xs8, c3, ky4]; iteration ((pt,z) merged, xsc24, ky).
            # (pt,z) merge is exact: rec stride 96 over P*4, w16 stride 4 over
            # P*4 (16 = 4*4).
            # 1) multiply by W16[z,ky] (bcast xs,c) -- bf16 2x
            nc.vector.tensor_tensor(
                _ap(rec[:], 0, [[P * RECE, 128], [96, P * 4], [4, 24], [1, 4]]),
                _ap(rec[:], 0, [[P * RECE, 128], [96, P * 4], [4, 24], [1, 4]]),
                _ap(w16[:], 0, [[P * 16, 128], [4, P * 4], [0, 24], [1, 4]]),
                mybir.AluOpType.mult)
            # 2) tree-reduce z (outer dim; fully packed)
            s192 = prods.tile([128, P * 192], BF16)
            nc.vector.tensor_tensor(
                _ap(s192[:], 0, [[P * 192, 128], [192, P], [96, 2], [1, 96]]),
                _ap(rec[:], 0, [[P * RECE, 128], [RECE, P], [96, 2], [1, 96]]),
                _ap(rec[:], 192, [[P * RECE, 128], [RECE, P], [96, 2], [1, 96]]),
                mybir.AluOpType.add)
            # tree-z L2 on PE: psum = I*s192_lo + I*s192_hi (f32 accumulate),
            # then Act drains psum -> s96 bf16.  4-point pieces (384 cols,
            # under the 512-col matmul ISA limit), 4 rotating psum banks.
            s96 = prods.tile([128, P * 96], BF16)
            NPC = 4           # points per piece
            for pc in range(P // NPC):
                psz = psums2.tile([128, NPC * 96], F32, name=f"psz{pc % 4}")
                nc.tensor.matmul(
                    psz[:],
                    id_sb[:],
                    _ap(s192[:], pc * NPC * 192,
                        [[P * 192, 128], [192, NPC], [1, 96]]),
                    start=True, stop=False)
                nc.tensor.matmul(
                    psz[:],
                    id_sb[:],
                    _ap(s192[:], pc * NPC * 192 + 96,
                        [[P * 192, 128], [192, NPC], [1, 96]]),
                    start=False, stop=True)
                nc.scalar.copy(
                    out=_ap(s96[:], pc * NPC * 96, [[P * 96, 128], [1, NPC * 96]]),
                    in_=psz[:])
            # 3) tree-reduce ky: s96[pt, xs8, c3, ky4] -> s24[pt, xs8, c3]
            #    L2 runs on gpsimd (stride-2 input is 1x on DVE anyway) and
            #    promotes to f32 for the remaining accumulation.
            s48 = prods.tile([128, P * 48], BF16)
            nc.vector.tensor_tensor(
                _ap(s48[:], 0, [[P * 48, 128], [48, P], [2, 24], [1, 2]]),
                _ap(s96[:], 0, [[P * 96, 128], [96, P], [4, 24], [1, 2]]),
                _ap(s96[:], 2, [[P * 96, 128], [96, P], [4, 24], [1, 2]]),
                mybir.AluOpType.add)
            s24 = prods.tile([128, P * 24], F32)
            nc.gpsimd.tensor_tensor(
                _ap(s24[:], 0, [[P * 24, 128], [24, P], [1, 24]]),
                _ap(s48[:], 0, [[P * 48, 128], [48, P], [2, 24]]),
                _ap(s48[:], 1, [[P * 48, 128], [48, P], [2, 24]]),
                mybir.AluOpType.add)
            # 4) multiply by W8c[xs,c] (f32 x bf16 -> f32)
            nc.vector.tensor_tensor(
                _ap(s24[:], 0, [[P * 24, 128], [1, P * 24]]),
                _ap(s24[:], 0, [[P * 24, 128], [1, P * 24]]),
                _ap(w8c[:], 0, [[P * 24, 128], [1, P * 24]]),
                mybir.AluOpType.mult)
            # 5) tree-reduce xs in f32: [xs8, c3] -> [c3]
            s12 = touts.tile([128, P * 12], F32)
            nc.vector.tensor_tensor(
                _ap(s12[:], 0, [[P * 12, 128], [12, P], [1, 12]]),
                _ap(s24[:], 0, [[P * 24, 128], [24, P], [1, 12]]),
                _ap(s24[:], 12, [[P * 24, 128], [24, P], [1, 12]]),
                mybir.AluOpType.add)
            s6 = touts.tile([128, P * 6], F32)
            nc.vector.tensor_tensor(
                _ap(s6[:], 0, [[P * 6, 128], [6, P], [1, 6]]),
                _ap(s12[:], 0, [[P * 12, 128], [12, P], [1, 6]]),
                _ap(s12[:], 6, [[P * 12, 128], [12, P], [1, 6]]),
                mybir.AluOpType.add)
            t_c = touts.tile([128, P * 3], F32)
            nc.vector.tensor_tensor(
                _ap(t_c[:], 0, [[P * 3, 128], [3, P], [1, 3]]),
                _ap(s6[:], 0, [[P * 6, 128], [6, P], [1, 3]]),
                _ap(s6[:], 3, [[P * 6, 128], [6, P], [1, 3]]),
                mybir.AluOpType.add)

            nc.sync.dma_start(
                out=t_out[:, ch * P:(ch + 1) * P, :],
                in_=t_c[:].rearrange("p (a b) -> p a b", b=3))


# ======================================================================
# Self-contained entry point: kernel(**inputs) -> np.ndarray
# ======================================================================

N_POINTS = 2_000_000
N_CORES = 8
PTS_PER_CORE = N_POINTS // N_CORES      # 250000
PAD_PER_CORE = 128 * COLS               # 253952

_CACHE = {}


def _build_nc(trace_sim=False, compile_=True):
    import concourse.bacc as bacc

    nc = bacc.Bacc(
        "TRN2",
        target_bir_lowering=False,
        debug=False,
        num_devices=N_CORES,
    )
    xs = nc.dram_tensor("xs", [128, COLS], F32, kind="ExternalInput").ap()
    ys = nc.dram_tensor("ys", [128, COLS], F32, kind="ExternalInput").ap()
    zs = nc.dram_tensor("zs", [128, COLS], F32, kind="ExternalInput").ap()
    phi = nc.dram_tensor("phi", [128, 128 * ZC], BF16, kind="ExternalInput").ap()
    t_out = nc.dram_tensor("t_out", [128, COLS, NC_], F32, kind="ExternalOutput").ap()

    with tile.TileContext(nc, trace_sim=trace_sim) as tc:
        bspline_kernel(tc, [t_out], [xs, ys, zs, phi])
    if compile_:
        nc.compile()
    return nc


def get_nc():
    if "nc" not in _CACHE:
        _CACHE["nc"] = _build_nc()
    return _CACHE["nc"]


def _shard(arr):
    """[N_POINTS] -> list of 8 [128, COLS] arrays (padded with zeros)."""
    out = []
    for c in range(N_CORES):
        s = arr[c * PTS_PER_CORE:(c + 1) * PTS_PER_CORE]
        p = np.zeros(PAD_PER_CORE, dtype=np.float32)
        p[:PTS_PER_CORE] = s
        out.append(p.reshape(128, COLS))
    return out


def _prep_phi(phi_x):
    """[128,128,128,3] f32 x-major -> [y, x*(z*c)] bf16 as uint16 view."""
    import ml_dtypes
    pt = np.ascontiguousarray(phi_x.transpose(1, 0, 2, 3)).reshape(128, 128 * ZC)
    return pt.astype(ml_dtypes.bfloat16).view(np.uint16)


def run_on_cores(x, y, z, phi_x, trace=False, **kw):
    from concourse.bass_utils import run_bass_kernel_spmd

    nc = get_nc()
    xsh, ysh, zsh = _shard(x), _shard(y), _shard(z)
    phi_r = _prep_phi(phi_x)
    in_maps = [
        {"xs": xsh[c], "ys": ysh[c], "zs": zsh[c], "phi": phi_r}
        for c in range(N_CORES)
    ]
    res = run_bass_kernel_spmd(
        nc, in_maps, core_ids=list(range(N_CORES)), trace=trace, **kw
    )
    outs = []
    for c in range(N_CORES):
        t = res.results[c]["t_out"].reshape(PAD_PER_CORE, NC_)
        outs.append(t[:PTS_PER_CORE])
    full = np.concatenate(outs, axis=0).astype(np.float32)
    return full, res


def kernel(x, y, z, phi_x):
    full, _ = run_on_cores(
        np.asarray(x, dtype=np.float32),
        np.asarray(y, dtype=np.float32),
        np.asarray(z, dtype=np.float32),
        np.asarray(phi_x, dtype=np.float32),
    )
    return full
